# revision 13
# baseline (speedup 1.0000x reference)
"""Trainium2 Bass kernel for nn_Explore_decoder_add (histogram_binning).

Strategy (8 NeuronCores, tensor-parallel on vocab), v2:
  - Wec is streamed as SINGLE-term bf16 (the 2e-2 absmax-rel gate leaves
    ~3 decades of precision headroom over the baseline's fp32-exact hi/lo
    scheme): halves HBM traffic and matmul count.
  - logits = h_t^T W0 + c_s^T W1 (+ bec + histogram penalty), all
    accumulated into 4 persistent PSUM banks laid out [p(128), chunk, b]:
      * W0 terms need only x[:,0,:] (an 8KB load) -> run early.
      * bec is injected via K=1 matmuls (lhsT=bec chunk, rhs=ones).
      * the seen-id histogram penalty is injected via one-hot matmuls
        (ohp scaled by -1e30) accumulating straight into PSUM.
      * W1 terms (after attention pooling produces c_s) stop each bank;
        the epilogue is then a single exp() per bank (scalar engine) to
        bf16, streamed out per bank.
  - Distributed softmax: per-core exp sums returned; host normalizes.
    No max-subtraction (logits are bounded ~|5|), no collectives.
  - Host pre-encodes layouts only: bf16 casts, x transposes (xT for the
    q matmul, xs for the c_s matmul, x0T), per-core shard coordinates
    (p = local_id % 128, c = local_id // 128, invalid -> 2^20).
  - Emission order = tile-scheduler priority: pooling chain first, ids/
    one-hot prep second, main accumulation last, so engines backfill idle
    slots without blocking the critical path.
"""

import numpy as np
import ml_dtypes

B, S, D = 16, 200, 128
V = 100000
NCORES = 8
VS = V // NCORES            # 12500 vocab per core
NCHUNK = 98                 # 98 chunks of 128
VSP = NCHUNK * 128          # 12544 padded shard width
NEG = -1.0e30
BIG = float(2 ** 20)        # invalid-id sentinel (c=8192 -> never matches)
BANKS = (25, 25, 25, 23)    # chunks per PSUM bank (sum = 98)

_prog_cache = {}


def _build_program():
    import concourse.bacc as bacc
    import concourse.mybir as mybir
    import concourse.tile as tile
    from concourse.masks import make_identity

    f32 = mybir.dt.float32
    bf16 = mybir.dt.bfloat16
    OP = mybir.AluOpType
    ACT = mybir.ActivationFunctionType

    nc = bacc.Bacc("TRN2", target_bir_lowering=False, debug=False,
                   num_devices=NCORES)

    # ---- I/O -------------------------------------------------------------
    x0T = nc.dram_tensor("x0T", (D, B), bf16, kind="ExternalInput").ap()
    xT = nc.dram_tensor("xT", (D, B * S), bf16, kind="ExternalInput").ap()
    xs0 = nc.dram_tensor("xs0", (128, B, D), bf16, kind="ExternalInput").ap()
    xs1 = nc.dram_tensor("xs1", (72, B, D), bf16, kind="ExternalInput").ap()
    wq = nc.dram_tensor("wq", (D, D), bf16, kind="ExternalInput").ap()
    wk = nc.dram_tensor("wk", (D, D), bf16, kind="ExternalInput").ap()
    wv = nc.dram_tensor("wv", (D, 1), bf16, kind="ExternalInput").ap()
    bq = nc.dram_tensor("bq", (D,), f32, kind="ExternalInput").ap()
    bk = nc.dram_tensor("bk", (D,), f32, kind="ExternalInput").ap()
    w0 = nc.dram_tensor("w0", (D, VSP), bf16, kind="ExternalInput").ap()
    w1 = nc.dram_tensor("w1", (D, VSP), bf16, kind="ExternalInput").ap()
    becp = nc.dram_tensor("becp", (1, VSP), bf16, kind="ExternalInput").ap()
    pT = nc.dram_tensor("pT", (128, 2 * B), f32, kind="ExternalInput").ap()
    cT = nc.dram_tensor("cT", (128, 2 * B), f32, kind="ExternalInput").ap()
    out = nc.dram_tensor("out", (128, NCHUNK * B), bf16,
                         kind="ExternalOutput").ap()
    sums_out = nc.dram_tensor("sums_out", (1, B), f32,
                              kind="ExternalOutput").ap()

    with tile.TileContext(nc) as tc:
        with (
            tc.tile_pool(name="sb", bufs=1) as sb,
            tc.tile_pool(name="oh", bufs=32) as oh,
            tc.tile_pool(name="pq", bufs=1, space="PSUM") as pq,
            tc.tile_pool(name="pp", bufs=1, space="PSUM") as pp,
        ):
            # ---- input DMAs: sync queue in stream order -----------------
            x0T_sb = sb.tile([D, B], bf16, name="x0T_sb")
            nc.sync.dma_start(out=x0T_sb[:, :], in_=x0T[:, :])
            wq_sb = sb.tile([D, D], bf16, name="wq_sb")
            nc.sync.dma_start(out=wq_sb[:, :], in_=wq[:, :])
            wk_sb = sb.tile([D, D], bf16, name="wk_sb")
            nc.sync.dma_start(out=wk_sb[:, :], in_=wk[:, :])
            wv_sb = sb.tile([D, 1], bf16, name="wv_sb")
            nc.sync.dma_start(out=wv_sb[:, :], in_=wv[:, :])
            bq_sb = sb.tile([D, 1], f32, name="bq_sb")
            nc.sync.dma_start(out=bq_sb[:, :], in_=bq[:, None])
            bk_sb = sb.tile([D, 1], f32, name="bk_sb")
            nc.sync.dma_start(out=bk_sb[:, :], in_=bk[:, None])
            xT_sb = sb.tile([D, B, S], bf16, name="xT_sb")
            xTf = xT_sb.rearrange("p b s -> p (b s)")
            for i in range(4):
                nc.sync.dma_start(out=xTf[:, i * 800:(i + 1) * 800],
                                  in_=xT[:, i * 800:(i + 1) * 800])
            xs0_sb = sb.tile([128, B, D], bf16, name="xs0_sb")
            nc.sync.dma_start(out=xs0_sb[:, :, :], in_=xs0[:, :, :])
            xs1_sb = sb.tile([128, B, D], bf16, name="xs1_sb")
            nc.sync.dma_start(out=xs1_sb[0:72, :, :], in_=xs1[:, :, :])
            w0_sb = sb.tile([D, VSP], bf16, name="w0_sb")
            w1_sb = sb.tile([D, VSP], bf16, name="w1_sb")
            for g in range(4):
                c0 = sum(BANKS[:g]) * 128
                c1 = c0 + BANKS[g] * 128
                nc.sync.dma_start(out=w0_sb[:, c0:c1], in_=w0[:, c0:c1])
            for g in range(4):
                c0 = sum(BANKS[:g]) * 128
                c1 = c0 + BANKS[g] * 128
                nc.sync.dma_start(out=w1_sb[:, c0:c1], in_=w1[:, c0:c1])

            # ---- small loads on the gpsimd (SWDGE) queue ----------------
            pT_sb = sb.tile([128, 2 * B], f32, name="pT_sb")
            nc.gpsimd.dma_start(out=pT_sb[:, :], in_=pT[:, :])
            cT_sb = sb.tile([128, 2 * B], f32, name="cT_sb")
            nc.gpsimd.dma_start(out=cT_sb[:, :], in_=cT[:, :])
            becp_sb = sb.tile([1, VSP], bf16, name="becp_sb")
            nc.gpsimd.dma_start(out=becp_sb[:, :], in_=becp[:, :])

            # ---- constants ----------------------------------------------
            ones_bf = sb.tile([1, B], bf16, name="ones_bf")
            nc.gpsimd.memset(ones_bf[:, :], 1.0)
            ones_col = sb.tile([128, 1], f32, name="ones_col")
            nc.gpsimd.memset(ones_col[:, :], 1.0)
            ones_colb = sb.tile([128, 1], bf16, name="ones_colb")
            nc.gpsimd.memset(ones_colb[:, :], 1.0)
            ones_row = sb.tile([1, 160], f32, name="ones_row")
            nc.gpsimd.memset(ones_row[:, :], 1.0)

            # ---- pooling chain (critical path; emitted first) ------------
            bias_eq = sb.tile([D, 1], f32, name="bias_eq")
            nc.vector.tensor_tensor(out=bias_eq[:, :], in0=bq_sb[:, :],
                                    in1=bk_sb[:, :], op=OP.add)
            pmisc1 = pp.tile([128, 512], f32, name="pmisc1", tag="misc1")
            pmisc2 = pp.tile([128, 512], f32, name="pmisc2", tag="misc2")
            pmisc3 = pp.tile([128, 512], f32, name="pmisc3", tag="misc3")
            kps = pmisc1[:, 0:B]
            nc.tensor.matmul(out=kps, lhsT=wk_sb[:, :],
                             rhs=x0T_sb[:, :], start=True, stop=True)
            kTb = sb.tile([128, B], f32, name="kTb")
            nc.vector.tensor_scalar(kTb[:, :], kps, bias_eq[:, 0:1],
                                    None, OP.add)

            # q/tanh per batch; scores computed TRANSPOSED [s, b] so the
            # pooling softmax sum runs on the PE (cross-partition ones
            # matmul) and 1/sum folds into v_cs at the end.
            fT = sb.tile([128, B, S], bf16, name="fT")
            scT0 = pmisc2[:, 0:B]
            scT1 = pmisc3[0:72, 2 * B:3 * B]
            qps2 = pq.tile([128, 2, S], f32, name="qps2", tag="q")
            for b in range(B):
                qsl = qps2[:, b % 2, :]
                nc.tensor.matmul(out=qsl, lhsT=wq_sb[:, :],
                                 rhs=xTf[:, b * S:(b + 1) * S],
                                 start=True, stop=True)
                nc.scalar.activation(out=fT[:, b, :], in_=qsl,
                                     func=ACT.Tanh, bias=kTb[:, b:b + 1])
                nc.tensor.matmul(out=scT0[:, b:b + 1],
                                 lhsT=fT[:, b, 0:128], rhs=wv_sb[:, :],
                                 start=(b == 0), stop=(b == B - 1))
                nc.tensor.matmul(out=scT1[:, b:b + 1],
                                 lhsT=fT[:, b, 128:200], rhs=wv_sb[:, :],
                                 start=(b == 0), stop=(b == B - 1))
            e_sT0 = sb.tile([128, B], bf16, name="e_sT0")
            nc.scalar.activation(out=e_sT0[:, :], in_=scT0,
                                 func=ACT.Exp)
            e_sT1 = sb.tile([128, B], bf16, name="e_sT1")
            nc.scalar.activation(out=e_sT1[0:72, :], in_=scT1,
                                 func=ACT.Exp)
            ssum_ps = pmisc3[0:1, 0:B]
            nc.tensor.matmul(out=ssum_ps, lhsT=ones_colb[:, :],
                             rhs=e_sT0[:, :], start=True, stop=False)
            nc.tensor.matmul(out=ssum_ps, lhsT=ones_colb[0:72, :],
                             rhs=e_sT1[0:72, :], start=False, stop=True)
            sinv_row = sb.tile([1, B], f32, name="sinv_row")
            nc.vector.reciprocal(sinv_row[:, :], ssum_ps)
            sinv_ps = pmisc3[:, B:2 * B]
            nc.tensor.matmul(out=sinv_ps, lhsT=ones_row[0:1, 0:128],
                             rhs=sinv_row[:, :], start=True, stop=True)
            sinv_sb = sb.tile([128, B], f32, name="sinv_sb")
            nc.vector.tensor_copy(sinv_sb[:, :], sinv_ps)

            csT = pmisc2[:, 2 * B:3 * B]
            for b in range(B):
                nc.tensor.matmul(out=csT[:, b:b + 1], lhsT=xs0_sb[:, b, :],
                                 rhs=e_sT0[:, b:b + 1], start=(b == 0),
                                 stop=False)
                nc.tensor.matmul(out=csT[:, b:b + 1],
                                 lhsT=xs1_sb[0:72, b, :],
                                 rhs=e_sT1[0:72, b:b + 1], start=False,
                                 stop=(b == B - 1))
            v_cs = sb.tile([128, B], bf16, name="v_cs")
            nc.vector.tensor_tensor(out=v_cs[:, :], in0=csT,
                                    in1=sinv_sb[:, :], op=OP.mult)

            # ---- ids -> one-hot prep (fills engine idle time) -----------
            iota_p_i = sb.tile([128, 128], mybir.dt.int32, name="iota_p_i")
            nc.gpsimd.iota(iota_p_i[:, :], pattern=[[1, 128]],
                           channel_multiplier=0)
            iota_c_i = sb.tile([128, NCHUNK], mybir.dt.int32, name="iota_c_i")
            nc.gpsimd.iota(iota_c_i[:, :], pattern=[[1, NCHUNK]],
                           channel_multiplier=0)
            iota_p = sb.tile([128, 128], bf16, name="iota_p")
            nc.vector.tensor_copy(iota_p[:, :], iota_p_i[:, :])
            iota_c = sb.tile([128, NCHUNK], bf16, name="iota_c")
            nc.vector.tensor_copy(iota_c[:, :], iota_c_i[:, :])

            oh_tiles = []
            for j in range(2 * B):
                ohp = oh.tile([128, 128], bf16, name="ohp", tag="ohp")
                nc.vector.tensor_scalar(ohp[:, :], iota_p[:, :],
                                        pT_sb[:, j:j + 1], NEG,
                                        OP.is_equal, OP.mult)
                ohc = oh.tile([128, NCHUNK], bf16, name="ohc", tag="ohc")
                nc.gpsimd.tensor_scalar(ohc[:, :], iota_c[:, :],
                                        cT_sb[:, j:j + 1], None,
                                        OP.is_equal)
                oh_tiles.append((ohp, ohc))

            # ---- main accumulation into 4 persistent PSUM banks ----------
            ps = []
            for g in range(4):
                ps.append(pp.tile([128, 32, B], f32, name=f"ps{g}",
                                  tag=f"ps{g}"))

            def bank_of(c):
                t = 0
                for g in range(4):
                    if c < t + BANKS[g]:
                        return g, c - t
                    t += BANKS[g]
                raise AssertionError

            # W0 terms (only need x0T; tiles stream in early).  After each
            # bank's W0 chunks, a filler matmul touches the bank's unused
            # tail so every byte leaves the pending-zero state before the
            # strided histogram matmuls.
            t = 0
            for g in range(4):
                nb = BANKS[g]
                for cl in range(nb):
                    c = t + cl
                    nc.tensor.matmul(out=ps[g][:, cl, :],
                                     lhsT=w0_sb[:, c * 128:(c + 1) * 128],
                                     rhs=x0T_sb[:, :], start=(cl == 0),
                                     stop=False)
                fill = ps[g][:, nb:32, :].rearrange("p c b -> p (c b)")
                nc.tensor.matmul(out=fill, lhsT=ones_row[0:1, 0:128],
                                 rhs=ones_row[0:1, 0:(32 - nb) * B],
                                 start=False, stop=False)
                t += nb
            # bec via K=1 matmuls
            for c in range(NCHUNK):
                g, cl = bank_of(c)
                nc.tensor.matmul(out=ps[g][:, cl, :],
                                 lhsT=becp_sb[0:1, c * 128:(c + 1) * 128],
                                 rhs=ones_bf[0:1, :], start=False,
                                 stop=False)
            # histogram penalty: ohp(-1e30 one-hot) x ohc per (batch, chunk)
            for j in range(2 * B):
                b = j % B
                ohp, ohc = oh_tiles[j]
                t = 0
                for g in range(4):
                    nb = BANKS[g]
                    nc.tensor.matmul(out=ps[g][:, 0:nb, b],
                                     lhsT=ohp[:, :],
                                     rhs=ohc[:, t:t + nb], start=False,
                                     stop=False)
                    t += nb
            # W1 terms close each bank; epilogue per bank
            exp_sb = sb.tile([128, NCHUNK, B], bf16, name="exp_sb")
            partial4 = sb.tile([128, 4, B], f32, name="partial4")
            outr = out.rearrange("p (c b) -> p c b", b=B)
            t = 0
            for g in range(4):
                nb = BANKS[g]
                for cl in range(nb):
                    c = t + cl
                    nc.tensor.matmul(out=ps[g][:, cl, :],
                                     lhsT=w1_sb[:, c * 128:(c + 1) * 128],
                                     rhs=v_cs[:, :], start=False,
                                     stop=(cl == nb - 1))
                gsl = slice(t, t + nb)
                nc.scalar.activation(out=exp_sb[:, gsl, :],
                                     in_=ps[g][:, 0:nb, :], func=ACT.Exp)
                nc.scalar.dma_start(out=outr[:, gsl, :],
                                    in_=exp_sb[:, gsl, :])
                nc.vector.tensor_reduce(
                    out=partial4[:, g, :],
                    in_=exp_sb[:, gsl, :].transpose([0, 2, 1]),
                    axis=mybir.AxisListType.X, op=OP.add)
                t += nb

            # ---- per-core softmax denominators --------------------------
            tot_ps = pmisc1[0:1, B:B + 4 * B]
            nc.tensor.matmul(out=tot_ps, lhsT=ones_col[:, :],
                             rhs=partial4.rearrange("p g b -> p (g b)"),
                             start=True, stop=True)
            sums_sb = sb.tile([1, B], f32, name="sums_sb")
            nc.vector.tensor_reduce(
                out=sums_sb[:, :],
                in_=tot_ps.rearrange("p (g b) -> p g b", g=4)
                .transpose([0, 2, 1]),
                axis=mybir.AxisListType.X, op=OP.add)
            nc.scalar.dma_start(out=sums_out[:, :], in_=sums_sb[:, :])

    nc.compile()
    return nc


def _get_program():
    if "nc" not in _prog_cache:
        _prog_cache["nc"] = _build_program()
    return _prog_cache["nc"]


def _host_inputs(x, x_ids, Wq, bq, Wk, bk, Wv, bv, Wec, bec):
    """Shared + per-core input arrays (host only re-encodes layouts)."""
    bf = ml_dtypes.bfloat16
    x = np.asarray(x, dtype=np.float32)
    ids = np.asarray(x_ids).astype(np.int64)
    xb = x.astype(bf)
    xT = np.ascontiguousarray(xb.transpose(2, 0, 1).reshape(D, B * S))
    x0T = np.ascontiguousarray(xb[:, 0, :].T)
    xs0 = np.ascontiguousarray(xb[:, 0:128, :].transpose(1, 0, 2))
    xs1 = np.ascontiguousarray(xb[:, 128:200, :].transpose(1, 0, 2))
    shared = {
        "x0T": x0T, "xT": xT, "xs0": xs0, "xs1": xs1,
        "wq": np.ascontiguousarray(np.asarray(Wq, np.float32).astype(bf)),
        "wk": np.ascontiguousarray(np.asarray(Wk, np.float32).astype(bf)),
        "wv": np.ascontiguousarray(np.asarray(Wv, np.float32).astype(bf)),
        "bq": np.ascontiguousarray(np.asarray(bq, np.float32)),
        "bk": np.ascontiguousarray(np.asarray(bk, np.float32)),
    }
    Wec = np.asarray(Wec, np.float32)
    bec = np.asarray(bec, np.float32)
    per_core = []
    for r in range(NCORES):
        lo, hi = r * VS, (r + 1) * VS
        wp = np.zeros((2 * D, VSP), np.float32)
        wp[:, :VS] = Wec[:, lo:hi]
        wpb = wp.astype(bf)
        bp = np.full((1, VSP), NEG, np.float32)
        bp[0, :VS] = bec[lo:hi]
        idl = ids - lo
        invalid = (ids < 2) | (idl < 0) | (idl >= VS)
        idl = np.where(invalid, int(BIG), idl)
        p = (idl % 128).astype(np.float32)
        c = (idl // 128).astype(np.float32)
        pTa = np.full((128, 2 * B), BIG, np.float32)
        cTa = np.full((128, 2 * B), BIG, np.float32)
        pTa[0:128, 0:B] = p[:, 0:128].T
        pTa[0:72, B:2 * B] = p[:, 128:200].T
        cTa[0:128, 0:B] = c[:, 0:128].T
        cTa[0:72, B:2 * B] = c[:, 128:200].T
        per_core.append({
            "w0": np.ascontiguousarray(wpb[0:D]),
            "w1": np.ascontiguousarray(wpb[D:2 * D]),
            "becp": np.ascontiguousarray(bp.astype(bf)),
            "pT": pTa, "cT": cTa,
        })
    return shared, per_core


def kernel(x, x_ids, Wq, bq, Wk, bk, Wv, bv, Wec, bec):
    shared, per_core = _host_inputs(x, x_ids, Wq, bq, Wk, bk, Wv, bv,
                                    Wec, bec)
    in_maps = [{**shared, **pc} for pc in per_core]

    nc = _get_program()
    from concourse.bass_utils import run_bass_kernel_spmd
    res = run_bass_kernel_spmd(nc, in_maps, core_ids=list(range(NCORES)))

    gsum = np.zeros((B,), np.float64)
    for r in range(NCORES):
        gsum += np.asarray(res.results[r]["sums_out"][0], np.float64)
    inv = (1.0 / gsum)[:, None].astype(np.float32)
    outp = np.empty((B, V), np.float32)
    for r in range(NCORES):
        o = np.asarray(res.results[r]["out"], np.float32)
        shard = o.reshape(128, NCHUNK, B).transpose(2, 1, 0).reshape(B, VSP)
        outp[:, r * VS:(r + 1) * VS] = shard[:, :VS] * inv
    return outp


# revision 14
# speedup vs baseline: 1.8224x; 1.8224x over previous
"""Trainium2 Bass kernel for nn_Explore_decoder_add (histogram_binning).

Strategy (8 NeuronCores, tensor-parallel on vocab), v2:
  - Wec is streamed as SINGLE-term bf16 (the 2e-2 absmax-rel gate leaves
    ~3 decades of precision headroom over the baseline's fp32-exact hi/lo
    scheme): halves HBM traffic and matmul count.
  - logits = h_t^T W0 + c_s^T W1 (+ bec + histogram penalty), all
    accumulated into 4 persistent PSUM banks laid out [p(128), chunk, b]:
      * W0 terms need only x[:,0,:] (an 8KB load) -> run early.
      * bec is injected via K=1 matmuls (lhsT=bec chunk, rhs=ones).
      * the seen-id histogram penalty is injected via one-hot matmuls
        (ohp scaled by -1e30) accumulating straight into PSUM.
      * W1 terms (after attention pooling produces c_s) stop each bank;
        the epilogue is then a single exp() per bank (scalar engine) to
        bf16, streamed out per bank.
  - Distributed softmax: per-core exp sums returned; host normalizes.
    No max-subtraction (logits are bounded ~|5|), no collectives.
  - Host pre-encodes layouts only: bf16 casts, x transposes (xT for the
    q matmul, xs for the c_s matmul, x0T), per-core shard coordinates
    (p = local_id % 128, c = local_id // 128, invalid -> 2^20).
  - Emission order = tile-scheduler priority: pooling chain first, ids/
    one-hot prep second, main accumulation last, so engines backfill idle
    slots without blocking the critical path.
"""

import numpy as np
import ml_dtypes

B, S, D = 16, 200, 128
V = 100000
NCORES = 8
VS = V // NCORES            # 12500 vocab per core
NCHUNK = 98                 # 98 chunks of 128
VSP = NCHUNK * 128          # 12544 padded shard width
NEG = -1.0e30
BIG = float(2 ** 20)        # invalid-id sentinel (c=8192 -> never matches)
BANKS = (25, 25, 25, 23)    # chunks per PSUM bank (sum = 98)

_prog_cache = {}


def _build_program():
    import concourse.bacc as bacc
    import concourse.mybir as mybir
    import concourse.tile as tile
    from concourse.masks import make_identity

    f32 = mybir.dt.float32
    bf16 = mybir.dt.bfloat16
    OP = mybir.AluOpType
    ACT = mybir.ActivationFunctionType

    nc = bacc.Bacc("TRN2", target_bir_lowering=False, debug=False,
                   num_devices=NCORES)

    # ---- I/O -------------------------------------------------------------
    packb = nc.dram_tensor("packb", (D, B + 2 * D + 1), bf16,
                           kind="ExternalInput").ap()
    packf = nc.dram_tensor("packf", (D, 2), f32, kind="ExternalInput").ap()
    xT = nc.dram_tensor("xT", (D, B * S), bf16, kind="ExternalInput").ap()
    xs0 = nc.dram_tensor("xs0", (128, B, D), bf16, kind="ExternalInput").ap()
    xs1 = nc.dram_tensor("xs1", (72, B, D), bf16, kind="ExternalInput").ap()
    w0 = nc.dram_tensor("w0", (D, VSP), bf16, kind="ExternalInput").ap()
    w1 = nc.dram_tensor("w1", (D, VSP), bf16, kind="ExternalInput").ap()
    becp = nc.dram_tensor("becp", (1, VSP), bf16, kind="ExternalInput").ap()
    pT = nc.dram_tensor("pT", (128, 2 * B), f32, kind="ExternalInput").ap()
    cT = nc.dram_tensor("cT", (128, 2 * B), f32, kind="ExternalInput").ap()
    out = nc.dram_tensor("out", (128, NCHUNK * B), bf16,
                         kind="ExternalOutput").ap()
    sums_out = nc.dram_tensor("sums_out", (1, B), f32,
                              kind="ExternalOutput").ap()

    with tile.TileContext(nc) as tc:
        with (
            tc.tile_pool(name="sb", bufs=1) as sb,
            tc.tile_pool(name="oh", bufs=32) as oh,
            tc.tile_pool(name="pq", bufs=1, space="PSUM") as pq,
            tc.tile_pool(name="pp", bufs=1, space="PSUM") as pp,
        ):
            # ---- input DMAs: sync queue in stream order -----------------
            packb_sb = sb.tile([D, B + 2 * D + 1], bf16, name="packb_sb")
            nc.sync.dma_start(out=packb_sb[:, :], in_=packb[:, :])
            packf_sb = sb.tile([D, 2], f32, name="packf_sb")
            nc.sync.dma_start(out=packf_sb[:, :], in_=packf[:, :])
            x0T_sb = packb_sb[:, 0:B]
            wq_sb = packb_sb[:, B:B + D]
            wk_sb = packb_sb[:, B + D:B + 2 * D]
            wv_sb = packb_sb[:, B + 2 * D:B + 2 * D + 1]
            bq_sb = packf_sb[:, 0:1]
            bk_sb = packf_sb[:, 1:2]
            xT_sb = sb.tile([D, B, S], bf16, name="xT_sb")
            xTf = xT_sb.rearrange("p b s -> p (b s)")
            for i in range(4):
                nc.sync.dma_start(out=xTf[:, i * 800:(i + 1) * 800],
                                  in_=xT[:, i * 800:(i + 1) * 800])
            xs0_sb = sb.tile([128, B, D], bf16, name="xs0_sb")
            nc.sync.dma_start(out=xs0_sb[:, :, :], in_=xs0[:, :, :])
            xs1_sb = sb.tile([128, B, D], bf16, name="xs1_sb")
            nc.sync.dma_start(out=xs1_sb[0:72, :, :], in_=xs1[:, :, :])
            w0_sb = sb.tile([D, VSP], bf16, name="w0_sb")
            w1_sb = sb.tile([D, VSP], bf16, name="w1_sb")
            for g in range(4):
                c0 = sum(BANKS[:g]) * 128
                c1 = c0 + BANKS[g] * 128
                nc.sync.dma_start(out=w0_sb[:, c0:c1], in_=w0[:, c0:c1])
            for g in range(4):
                c0 = sum(BANKS[:g]) * 128
                c1 = c0 + BANKS[g] * 128
                nc.sync.dma_start(out=w1_sb[:, c0:c1], in_=w1[:, c0:c1])

            # ---- small loads on the gpsimd (SWDGE) queue ----------------
            pT_sb = sb.tile([128, 2 * B], f32, name="pT_sb")
            nc.gpsimd.dma_start(out=pT_sb[:, :], in_=pT[:, :])
            cT_sb = sb.tile([128, 2 * B], f32, name="cT_sb")
            nc.gpsimd.dma_start(out=cT_sb[:, :], in_=cT[:, :])
            becp_sb = sb.tile([1, VSP], bf16, name="becp_sb")
            nc.gpsimd.dma_start(out=becp_sb[:, :], in_=becp[:, :])

            # ---- constants ----------------------------------------------
            ones_bf = sb.tile([1, B], bf16, name="ones_bf")
            nc.gpsimd.memset(ones_bf[:, :], 1.0)
            ones_col = sb.tile([128, 1], f32, name="ones_col")
            nc.gpsimd.memset(ones_col[:, :], 1.0)
            ones_colb = sb.tile([128, 1], bf16, name="ones_colb")
            nc.gpsimd.memset(ones_colb[:, :], 1.0)
            ones_row = sb.tile([1, 160], f32, name="ones_row")
            nc.gpsimd.memset(ones_row[:, :], 1.0)

            # ---- pooling chain (critical path; emitted first) ------------
            bias_eq = sb.tile([D, 1], f32, name="bias_eq")
            nc.vector.tensor_tensor(out=bias_eq[:, :], in0=bq_sb,
                                    in1=bk_sb, op=OP.add)
            pmisc1 = pp.tile([128, 512], f32, name="pmisc1", tag="misc1")
            pmisc2 = pp.tile([128, 512], f32, name="pmisc2", tag="misc2")
            pmisc3 = pp.tile([128, 512], f32, name="pmisc3", tag="misc3")
            kps = pmisc1[:, 0:B]
            nc.tensor.matmul(out=kps, lhsT=wk_sb,
                             rhs=x0T_sb, start=True, stop=True)
            kTb = sb.tile([128, B], f32, name="kTb")
            nc.vector.tensor_scalar(kTb[:, :], kps, bias_eq[:, 0:1],
                                    None, OP.add)

            # q/tanh per batch; scores computed TRANSPOSED [s, b] so the
            # pooling softmax sum runs on the PE (cross-partition ones
            # matmul) and 1/sum folds into v_cs at the end.
            fT = sb.tile([128, B, S], bf16, name="fT")
            scT0 = pmisc2[:, 0:B]
            scT1 = pmisc3[0:72, 2 * B:3 * B]
            qps2 = pq.tile([128, 2, S], f32, name="qps2", tag="q")
            for b in range(B):
                qsl = qps2[:, b % 2, :]
                nc.tensor.matmul(out=qsl, lhsT=wq_sb,
                                 rhs=xTf[:, b * S:(b + 1) * S],
                                 start=True, stop=True)
                nc.scalar.activation(out=fT[:, b, :], in_=qsl,
                                     func=ACT.Tanh, bias=kTb[:, b:b + 1])
                nc.tensor.matmul(out=scT0[:, b:b + 1],
                                 lhsT=fT[:, b, 0:128], rhs=wv_sb,
                                 start=(b == 0), stop=(b == B - 1))
                nc.tensor.matmul(out=scT1[:, b:b + 1],
                                 lhsT=fT[:, b, 128:200], rhs=wv_sb,
                                 start=(b == 0), stop=(b == B - 1))
            e_sT0 = sb.tile([128, B], bf16, name="e_sT0")
            nc.scalar.activation(out=e_sT0[:, :], in_=scT0,
                                 func=ACT.Exp)
            e_sT1 = sb.tile([128, B], bf16, name="e_sT1")
            nc.scalar.activation(out=e_sT1[0:72, :], in_=scT1,
                                 func=ACT.Exp)
            ssum_ps = pmisc3[0:1, 0:B]
            nc.tensor.matmul(out=ssum_ps, lhsT=ones_colb[:, :],
                             rhs=e_sT0[:, :], start=True, stop=False)
            nc.tensor.matmul(out=ssum_ps, lhsT=ones_colb[0:72, :],
                             rhs=e_sT1[0:72, :], start=False, stop=True)
            sinv_row = sb.tile([1, B], f32, name="sinv_row")
            nc.vector.reciprocal(sinv_row[:, :], ssum_ps)
            sinv_ps = pmisc3[:, B:2 * B]
            nc.tensor.matmul(out=sinv_ps, lhsT=ones_row[0:1, 0:128],
                             rhs=sinv_row[:, :], start=True, stop=True)
            sinv_sb = sb.tile([128, B], f32, name="sinv_sb")
            nc.vector.tensor_copy(sinv_sb[:, :], sinv_ps)

            csT = pmisc2[:, 2 * B:3 * B]
            for b in range(B):
                nc.tensor.matmul(out=csT[:, b:b + 1], lhsT=xs0_sb[:, b, :],
                                 rhs=e_sT0[:, b:b + 1], start=(b == 0),
                                 stop=False)
                nc.tensor.matmul(out=csT[:, b:b + 1],
                                 lhsT=xs1_sb[0:72, b, :],
                                 rhs=e_sT1[0:72, b:b + 1], start=False,
                                 stop=(b == B - 1))
            v_cs = sb.tile([128, B], bf16, name="v_cs")
            nc.vector.tensor_tensor(out=v_cs[:, :], in0=csT,
                                    in1=sinv_sb[:, :], op=OP.mult)

            # ---- ids -> one-hot prep (fills engine idle time) -----------
            iota_p_i = sb.tile([128, 128], mybir.dt.int32, name="iota_p_i")
            nc.gpsimd.iota(iota_p_i[:, :], pattern=[[1, 128]],
                           channel_multiplier=0)
            iota_c_i = sb.tile([128, NCHUNK], mybir.dt.int32, name="iota_c_i")
            nc.gpsimd.iota(iota_c_i[:, :], pattern=[[1, NCHUNK]],
                           channel_multiplier=0)
            iota_p = sb.tile([128, 128], bf16, name="iota_p")
            nc.vector.tensor_copy(iota_p[:, :], iota_p_i[:, :])
            iota_c = sb.tile([128, NCHUNK], bf16, name="iota_c")
            nc.vector.tensor_copy(iota_c[:, :], iota_c_i[:, :])

            oh_tiles = []
            for j in range(2 * B):
                ohp = oh.tile([128, 128], bf16, name="ohp", tag="ohp")
                nc.vector.tensor_scalar(ohp[:, :], iota_p[:, :],
                                        pT_sb[:, j:j + 1], NEG,
                                        OP.is_equal, OP.mult)
                ohc = oh.tile([128, NCHUNK], bf16, name="ohc", tag="ohc")
                nc.vector.tensor_scalar(ohc[:, :], iota_c[:, :],
                                        cT_sb[:, j:j + 1], None,
                                        OP.is_equal)
                oh_tiles.append((ohp, ohc))

            # ---- main accumulation into 4 persistent PSUM banks ----------
            ps = []
            for g in range(4):
                ps.append(pp.tile([128, 32, B], f32, name=f"ps{g}",
                                  tag=f"ps{g}"))

            def bank_of(c):
                t = 0
                for g in range(4):
                    if c < t + BANKS[g]:
                        return g, c - t
                    t += BANKS[g]
                raise AssertionError

            # W0 terms (only need x0T; tiles stream in early).  After each
            # bank's W0 chunks, a filler matmul touches the bank's unused
            # tail so every byte leaves the pending-zero state before the
            # strided histogram matmuls.
            t = 0
            for g in range(4):
                nb = BANKS[g]
                for cl in range(nb):
                    c = t + cl
                    nc.tensor.matmul(out=ps[g][:, cl, :],
                                     lhsT=w0_sb[:, c * 128:(c + 1) * 128],
                                     rhs=x0T_sb, start=(cl == 0),
                                     stop=False)
                fill = ps[g][:, nb:32, :].rearrange("p c b -> p (c b)")
                nc.tensor.matmul(out=fill, lhsT=ones_row[0:1, 0:128],
                                 rhs=ones_row[0:1, 0:(32 - nb) * B],
                                 start=False, stop=False)
                t += nb
            # bec via K=1 matmuls
            for c in range(NCHUNK):
                g, cl = bank_of(c)
                nc.tensor.matmul(out=ps[g][:, cl, :],
                                 lhsT=becp_sb[0:1, c * 128:(c + 1) * 128],
                                 rhs=ones_bf[0:1, :], start=False,
                                 stop=False)
            # histogram penalty: ohp(-1e30 one-hot) x ohc per (batch, chunk)
            for j in range(2 * B):
                b = j % B
                ohp, ohc = oh_tiles[j]
                t = 0
                for g in range(4):
                    nb = BANKS[g]
                    nc.tensor.matmul(out=ps[g][:, 0:nb, b],
                                     lhsT=ohp[:, :],
                                     rhs=ohc[:, t:t + nb], start=False,
                                     stop=False)
                    t += nb
            # W1 terms close each bank; epilogue per bank
            exp_sb = sb.tile([128, NCHUNK, B], bf16, name="exp_sb")
            partial4 = sb.tile([128, 4, B], f32, name="partial4")
            outr = out.rearrange("p (c b) -> p c b", b=B)
            t = 0
            for g in range(4):
                nb = BANKS[g]
                for cl in range(nb):
                    c = t + cl
                    nc.tensor.matmul(out=ps[g][:, cl, :],
                                     lhsT=w1_sb[:, c * 128:(c + 1) * 128],
                                     rhs=v_cs[:, :], start=False,
                                     stop=(cl == nb - 1))
                gsl = slice(t, t + nb)
                nc.scalar.activation(out=exp_sb[:, gsl, :],
                                     in_=ps[g][:, 0:nb, :], func=ACT.Exp)
                nc.scalar.dma_start(out=outr[:, gsl, :],
                                    in_=exp_sb[:, gsl, :])
                nc.vector.tensor_reduce(
                    out=partial4[:, g, :],
                    in_=exp_sb[:, gsl, :].transpose([0, 2, 1]),
                    axis=mybir.AxisListType.X, op=OP.add)
                t += nb

            # ---- per-core softmax denominators --------------------------
            tot_ps = pmisc1[0:1, B:B + 4 * B]
            nc.tensor.matmul(out=tot_ps, lhsT=ones_col[:, :],
                             rhs=partial4.rearrange("p g b -> p (g b)"),
                             start=True, stop=True)
            sums_sb = sb.tile([1, B], f32, name="sums_sb")
            nc.vector.tensor_reduce(
                out=sums_sb[:, :],
                in_=tot_ps.rearrange("p (g b) -> p g b", g=4)
                .transpose([0, 2, 1]),
                axis=mybir.AxisListType.X, op=OP.add)
            nc.scalar.dma_start(out=sums_out[:, :], in_=sums_sb[:, :])

    nc.compile()
    return nc


def _get_program():
    if "nc" not in _prog_cache:
        _prog_cache["nc"] = _build_program()
    return _prog_cache["nc"]


def _host_inputs(x, x_ids, Wq, bq, Wk, bk, Wv, bv, Wec, bec):
    """Shared + per-core input arrays (host only re-encodes layouts)."""
    bf = ml_dtypes.bfloat16
    x = np.asarray(x, dtype=np.float32)
    ids = np.asarray(x_ids).astype(np.int64)
    xb = x.astype(bf)
    xT = np.ascontiguousarray(xb.transpose(2, 0, 1).reshape(D, B * S))
    x0T = np.ascontiguousarray(xb[:, 0, :].T)
    xs0 = np.ascontiguousarray(xb[:, 0:128, :].transpose(1, 0, 2))
    xs1 = np.ascontiguousarray(xb[:, 128:200, :].transpose(1, 0, 2))
    packb = np.concatenate([
        x0T,
        np.asarray(Wq, np.float32).astype(bf),
        np.asarray(Wk, np.float32).astype(bf),
        np.asarray(Wv, np.float32).astype(bf),
    ], axis=1)
    packf = np.stack([np.asarray(bq, np.float32),
                      np.asarray(bk, np.float32)], axis=1)
    shared = {
        "packb": np.ascontiguousarray(packb),
        "packf": np.ascontiguousarray(packf),
        "xT": xT, "xs0": xs0, "xs1": xs1,
    }
    Wec = np.asarray(Wec, np.float32)
    bec = np.asarray(bec, np.float32)
    per_core = []
    for r in range(NCORES):
        lo, hi = r * VS, (r + 1) * VS
        wp = np.zeros((2 * D, VSP), np.float32)
        wp[:, :VS] = Wec[:, lo:hi]
        wpb = wp.astype(bf)
        bp = np.full((1, VSP), NEG, np.float32)
        bp[0, :VS] = bec[lo:hi]
        idl = ids - lo
        invalid = (ids < 2) | (idl < 0) | (idl >= VS)
        idl = np.where(invalid, int(BIG), idl)
        p = (idl % 128).astype(np.float32)
        c = (idl // 128).astype(np.float32)
        pTa = np.full((128, 2 * B), BIG, np.float32)
        cTa = np.full((128, 2 * B), BIG, np.float32)
        pTa[0:128, 0:B] = p[:, 0:128].T
        pTa[0:72, B:2 * B] = p[:, 128:200].T
        cTa[0:128, 0:B] = c[:, 0:128].T
        cTa[0:72, B:2 * B] = c[:, 128:200].T
        per_core.append({
            "w0": np.ascontiguousarray(wpb[0:D]),
            "w1": np.ascontiguousarray(wpb[D:2 * D]),
            "becp": np.ascontiguousarray(bp.astype(bf)),
            "pT": pTa, "cT": cTa,
        })
    return shared, per_core


def kernel(x, x_ids, Wq, bq, Wk, bk, Wv, bv, Wec, bec):
    shared, per_core = _host_inputs(x, x_ids, Wq, bq, Wk, bk, Wv, bv,
                                    Wec, bec)
    in_maps = [{**shared, **pc} for pc in per_core]

    nc = _get_program()
    from concourse.bass_utils import run_bass_kernel_spmd
    res = run_bass_kernel_spmd(nc, in_maps, core_ids=list(range(NCORES)))

    gsum = np.zeros((B,), np.float64)
    for r in range(NCORES):
        gsum += np.asarray(res.results[r]["sums_out"][0], np.float64)
    inv = (1.0 / gsum)[:, None].astype(np.float32)
    outp = np.empty((B, V), np.float32)
    for r in range(NCORES):
        o = np.asarray(res.results[r]["out"], np.float32)
        shard = o.reshape(128, NCHUNK, B).transpose(2, 1, 0).reshape(B, VSP)
        outp[:, r * VS:(r + 1) * VS] = shard[:, :VS] * inv
    return outp
